# revision 1
# baseline (speedup 1.0000x reference)
"""MLA/MQA attention block (nn_Attention_33406255628587) on 8 Trainium2 cores.

Sharding: data-parallel over batch (4) x tensor-parallel over query heads
(16 -> 2 groups of 8).  Core c handles batch c//2, head group c%2.  All 8
cores run one SPMD program; the host sums the two o_proj partials of each
batch pair (o_proj contracts over heads, so head-split partials add).

Device strategy (all matmuls fp32r: 1 cycle/row at N>=256, fp32 PSUM):
  - hidden^T arrives host-transposed so D (contraction dim of both
    down-projections) sits on SBUF partitions.
  - q_lat/kv_lat are produced TRANSPOSED ([R, T]); RMS-norm across
    partitions uses a ones-matrix matmul (the sum of squares lands
    broadcast across all partitions, so the per-column 1/rms multiplies
    directly, no transposes needed).
  - up-projections give q^T / k^T with head_dim on partitions: per-head
    q^T is one 128-partition tile; RoPE + nope/rope gating fold to
    [q1*C1 - q2*S1 ; q2*C1 + q1*S1] with C1 = a+(1-a)*cos_blend,
    S1 = (1-a)*sin_blend precomputed once.
  - attention runs fully transposed: S^T[k, q] = kf^T_chunk.T @ qf^T.
    No max-subtraction (|scores*SCALE| is small for this problem; masked
    entries underflow exp to exact 0): exp on ACT straight out of PSUM
    (SCALE folded in, per-key padding bias as the per-partition bias AP,
    causal triangles added only on the two diagonal key-chunks,
    fully-masked key-chunks skipped entirely); sum_k exp via a
    ones-column matmul; 1/sum broadcast across partitions via a rank-1
    matmul.
  - attn output accumulates transposed [d, q] (lhsT = v chunks), which is
    exactly the lhsT layout o_proj needs.
"""

import sys

sys.path.insert(0, "/opt/trn_rl_repo")

import numpy as np

B, T, D, H, HD = 4, 1024, 2048, 16, 128
QR, KVR = 1536, 512
EPS = 1e-6
NEG = -1000000000.0
SCALE = HD ** -0.5

P = 128
H2 = HD // 2      # 64
HH = H // 2       # 8 heads per core
QB = 256          # query block in attention
NQB = T // QB     # 4
KT = T // P       # 8 key chunks
DK = D // P       # 16
QRK = QR // P     # 12
KVK = KVR // P    # 4
TN = T // 512     # 2

_nc_cache = {}


def build_kernel(dbg=False, use_pad=False):
    import concourse.bacc as bacc
    import concourse.tile as tile
    from concourse import mybir
    from contextlib import ExitStack

    F32 = mybir.dt.float32
    F32R = mybir.dt.float32r
    AF = mybir.ActivationFunctionType
    mul = mybir.AluOpType.mult
    add = mybir.AluOpType.add
    sub = mybir.AluOpType.subtract

    nc = bacc.Bacc("TRN2", target_bir_lowering=False, debug=False)

    # ---- DRAM I/O (host-prepared layouts, see _prep_core_inputs) ----
    hid = nc.dram_tensor("hid", [DK, P, T], F32R, kind="ExternalInput")
    wqa = nc.dram_tensor("wqa", [QRK, P, DK, P], F32R, kind="ExternalInput")
    wkva = nc.dram_tensor("wkva", [KVK, P, DK, P], F32R, kind="ExternalInput")
    wqb = nc.dram_tensor("wqb", [HH, P, QRK, P], F32R, kind="ExternalInput")
    wkvb = nc.dram_tensor("wkvb", [2, P, KVK, P], F32R, kind="ExternalInput")
    wo = nc.dram_tensor("wo", [4, HH, P, 512], F32R, kind="ExternalInput")
    trig = nc.dram_tensor("trig", [4, H2, T], F32, kind="ExternalInput")
    gates = nc.dram_tensor("gates", [1, 2], F32, kind="ExternalInput")
    padb = nc.dram_tensor("padb", [1, T], F32R, kind="ExternalInput")
    dmask = nc.dram_tensor("dmask", [P, 2, QB], F32, kind="ExternalInput")
    onesm = nc.dram_tensor("onesm", [P, QB], F32R, kind="ExternalInput")
    o_part = nc.dram_tensor("o_part", [T, D], F32, kind="ExternalOutput")
    if dbg:
        dbg_qnT = nc.dram_tensor("dbg_qnT", [P, QRK, T], F32, kind="ExternalOutput")
        dbg_kvnT = nc.dram_tensor("dbg_kvnT", [P, KVK, T], F32, kind="ExternalOutput")
        dbg_qfT = nc.dram_tensor("dbg_qfT", [P, HH, T], F32, kind="ExternalOutput")
        dbg_kfT = nc.dram_tensor("dbg_kfT", [P, T], F32, kind="ExternalOutput")
        dbg_v = nc.dram_tensor("dbg_v", [P, KT, P], F32, kind="ExternalOutput")
        dbg_outT = nc.dram_tensor("dbg_outT", [P, HH, T], F32, kind="ExternalOutput")
        dbg_C1 = nc.dram_tensor("dbg_C1", [P, T], F32, kind="ExternalOutput")
        dbg_S1 = nc.dram_tensor("dbg_S1", [P, T], F32, kind="ExternalOutput")

    with tile.TileContext(nc, pool_alloc_mode="queue") as tc, ExitStack() as top:
        # ---------- persistent pools ----------
        consts = top.enter_context(tc.tile_pool(name="consts", bufs=1))
        latp = top.enter_context(tc.tile_pool(name="latp", bufs=1))

        padr = consts.tile([1, T], F32R)
        nc.sync.dma_start(padr[:], padb[:])
        dm_sb = consts.tile([P, 2, QB], F32)
        nc.sync.dma_start(dm_sb[:], dmask[:])
        ones_mat = consts.tile([P, QB], F32R)
        nc.sync.dma_start(ones_mat[:], onesm[:])
        ones_col = ones_mat[:, 0:1]
        ones_row = ones_mat[0:1, 0:P]
        ones_row2 = ones_mat[0:1, :]
        C1 = consts.tile([P, T], F32)
        S1 = consts.tile([P, T], F32)
        eps_sb = consts.tile([P, 1], F32)
        nc.vector.memset(eps_sb[:], EPS)

        qnT = latp.tile([P, QRK, T], F32R)     # 48KB/p
        kvnT = latp.tile([P, KVK, T], F32R)    # 16KB/p

        # ---------- hidden^T tiles (DMAs emitted inside down_proj so the
        # first weight chunks land first and PE tracks hid arrival) ----------
        hctx = ExitStack()
        hidp = hctx.enter_context(tc.tile_pool(name="hidp", bufs=1))
        normp = hctx.enter_context(tc.tile_pool(name="normp", bufs=1))
        hid_sb = [hidp.tile([P, T], F32R, name=f"hid_{k}") for k in range(DK)]

        _hid_emitted = set()

        def emit_hid_dmas(which):
            ks = range(DK) if which is None else [which]
            for k in ks:
                if k not in _hid_emitted:
                    _hid_emitted.add(k)
                    nc.scalar.dma_start(hid_sb[k][:], hid[k])

        # ---------- down-projections (transposed) + RMS-norm ----------
        # qnT stays UNSCALED: its 1/rms (per T column) is folded into the
        # RoPE tables (C1q/S1q) since the up-projection + RoPE are linear
        # per T column.  kvnT is scaled in place (v needs it too).
        C1q = consts.tile([P, T], F32)
        S1q = consts.tile([P, T], F32)
        if True:
            def down_proj(latT, nchunks, w_pre, name, n_pre=0,
                          post_w_hook=None):
                with tc.tile_pool(name=f"w_{name}", bufs=max(3, n_pre)) as wp, \
                     tc.tile_pool(name=f"sq_{name}", bufs=2) as sqp, \
                     tc.tile_pool(name=f"ps_{name}", bufs=max(2, n_pre),
                                  space="PSUM") as psp, \
                     tc.tile_pool(name=f"pss_{name}", bufs=1,
                                  space="PSUM") as pssp:
                    ss = pssp.tile([P, TN, 512], F32)

                    def finish_m(m, ps_m):
                        nc.scalar.copy(latT[:, m, :],
                                       ps_m[:].rearrange("p a b -> p (a b)"))
                        sq = sqp.tile([P, T], F32R, tag="sq",
                                      name=f"sq_{name}_{m}")
                        nc.scalar.square(sq[:], latT[:, m, :].bitcast(F32))
                        for tn in range(TN):
                            nc.tensor.matmul(
                                ss[:, tn, :], ones_mat[:, 0:P],
                                sq[:, tn * 512:(tn + 1) * 512],
                                start=(m == 0), stop=(m == nchunks - 1))

                    # first n_pre chunks: k-outer, so the matmuls consume the
                    # streaming hid chunks as they land instead of stalling
                    if n_pre:
                        w_pres, ps_pre = [], []
                        for m in range(n_pre):
                            if post_w_hook is not None:
                                post_w_hook(m)      # hid[m] DMA
                            w_m = wp.tile([P, DK, P], F32R, tag="w",
                                          name=f"w_{name}_p{m}")
                            nc.sync.dma_start(w_m[:], w_pre[m])
                            w_pres.append(w_m)
                            ps_pre.append(psp.tile([P, TN, 512], F32, tag="ps",
                                                   name=f"ps_{name}_p{m}"))
                        if post_w_hook is not None:
                            post_w_hook(None)       # remaining hid DMAs
                        for k in range(DK):
                            for m in range(n_pre):
                                for tn in range(TN):
                                    ts = slice(tn * 512, (tn + 1) * 512)
                                    nc.tensor.matmul(
                                        ps_pre[m][:, tn, :], w_pres[m][:, k, :],
                                        hid_sb[k][:, ts],
                                        start=(k == 0), stop=(k == DK - 1))
                        for m in range(n_pre):
                            finish_m(m, ps_pre[m])

                    for m in range(n_pre, nchunks):
                        w_m = wp.tile([P, DK, P], F32R, tag="w")
                        nc.sync.dma_start(w_m[:], w_pre[m])
                        ps = psp.tile([P, TN, 512], F32, tag="ps")
                        for tn in range(TN):
                            ts = slice(tn * 512, (tn + 1) * 512)
                            for k in range(DK):
                                nc.tensor.matmul(
                                    ps[:, tn, :], w_m[:, k, :], hid_sb[k][:, ts],
                                    start=(k == 0), stop=(k == DK - 1))
                        finish_m(m, ps)
                    # rs = 1/sqrt(mean(sq)+eps), already partition-broadcast
                    rsb = normp.tile([P, T], F32, name=f"rsb_{name}")
                    for tn in range(TN):
                        nc.scalar.activation(
                            rsb[:, tn * 512:(tn + 1) * 512], ss[:, tn, :],
                            AF.Sqrt, bias=eps_sb[:], scale=1.0 / (nchunks * P))
                    nc.vector.reciprocal(rsb[:], rsb[:])
                    return rsb

            rsb_qa = down_proj(qnT, QRK, wqa, "qa", n_pre=3,
                               post_w_hook=emit_hid_dmas)
            # ---------- gates + blended RoPE tables ----------
            with tc.tile_pool(name="trigp", bufs=1) as trigp, \
                 tc.tile_pool(name="ps_g", bufs=1, space="PSUM") as ps_g:
                g_sb = trigp.tile([1, 2], F32)
                nc.sync.dma_start(g_sb[:], gates[:])
                sig = trigp.tile([1, 2], F32R)     # (a, g)
                nc.scalar.activation(sig[:], g_sb[:], AF.Sigmoid)
                isig = trigp.tile([1, 2], F32R)    # (1-a, 1-g)
                nc.scalar.activation(isig[:], sig[:].bitcast(F32), AF.Identity,
                                     bias=1.0, scale=-1.0)
                # broadcast the 4 scalars to 64 partitions: s4 = [64, 4]
                ps4 = ps_g.tile([H2, 4], F32)
                nc.tensor.matmul(ps4[:, 0:2], ones_row[:, 0:H2], sig[:],
                                 start=True, stop=True)
                nc.tensor.matmul(ps4[:, 2:4], ones_row[:, 0:H2], isig[:],
                                 start=True, stop=True)
                s4 = trigp.tile([H2, 4], F32)
                nc.scalar.copy(s4[:], ps4[:])
                a_c, g_c = s4[:, 0:1], s4[:, 1:2]
                ia_c, ig_c = s4[:, 2:3], s4[:, 3:4]

                tg = trigp.tile([H2, 4, T], F32)
                for j in range(4):
                    nc.gpsimd.dma_start(tg[:, j, :], trig[j])
                cb = trigp.tile([H2, T], F32)
                sb2 = trigp.tile([H2, T], F32)
                # cos_blend = g*cos_g + (1-g)*cos_l   (tg: cg, cl, sg, sl)
                nc.vector.tensor_scalar(out=cb[:], in0=tg[:, 0, :], scalar1=g_c,
                                        scalar2=None, op0=mul)
                nc.vector.scalar_tensor_tensor(out=cb[:], in0=tg[:, 1, :], scalar=ig_c,
                                               in1=cb[:], op0=mul, op1=add)
                nc.vector.tensor_scalar(out=sb2[:], in0=tg[:, 2, :], scalar1=g_c,
                                        scalar2=None, op0=mul)
                nc.vector.scalar_tensor_tensor(out=sb2[:], in0=tg[:, 3, :], scalar=ig_c,
                                               in1=sb2[:], op0=mul, op1=add)
                # C1 = (1-a)*cos_blend + a ; S1 = (1-a)*sin_blend
                # duplicated into both partition halves so RoPE runs full-width
                for off in (0, H2):
                    nc.vector.tensor_scalar(out=C1[off:off + H2, :], in0=cb[:],
                                            scalar1=ia_c, scalar2=a_c,
                                            op0=mul, op1=add)
                    nc.vector.tensor_scalar(out=S1[off:off + H2, :], in0=sb2[:],
                                            scalar1=ia_c, scalar2=None, op0=mul)


            nc.vector.tensor_tensor(C1q[:], C1[:], rsb_qa[:], mul)
            nc.vector.tensor_tensor(S1q[:], S1[:], rsb_qa[:], mul)
            rsb_kva = down_proj(kvnT, KVK, wkva, "kva")
            for m in range(KVK):
                nc.gpsimd.tensor_tensor(
                    kvnT[:, m, :], kvnT[:, m, :].bitcast(F32),
                    rsb_kva[:], mul)
            hctx.close()
            if dbg:
                nc.sync.dma_start(dbg_qnT[:], qnT[:].bitcast(F32))  # UNSCALED now
                nc.sync.dma_start(dbg_kvnT[:], kvnT[:].bitcast(F32))
                nc.sync.dma_start(dbg_C1[:], C1[:])
                nc.sync.dma_start(dbg_S1[:], S1[:])

        # ---------- up-projections + RoPE/gating ----------
        qfp = top.enter_context(tc.tile_pool(name="qfp", bufs=1))
        qfT = qfp.tile([P, HH, T], F32R)       # 32KB/p

        def rope_gate(dst, ps, ts, Ct, St):
            # dst[0:64]  = p1*Ct - p2*St ; dst[64:128] = p2*Ct + p1*St
            # Ct/St are partition-duplicated.  tb is written with its halves
            # pre-swapped (PSUM+SB operands may differ in base partition;
            # SB+SB operands of the combine ops must match).
            W = ps.shape[-1]
            ta = rtmp.tile([P, T], F32, tag="ta", name="rta")[:, :W]
            tb = rtmp.tile([P, T], F32, tag="tb", name="rtb")[:, :W]
            nc.vector.tensor_tensor(ta[:], ps[:], Ct[:, ts], mul)
            nc.vector.tensor_tensor(tb[0:H2, :], ps[H2:P, :], St[0:H2, ts], mul)
            nc.vector.tensor_tensor(tb[H2:P, :], ps[0:H2, :], St[H2:P, ts], mul)
            nc.vector.tensor_tensor(dst[0:H2, :], ta[0:H2, :], tb[0:H2, :], sub)
            nc.vector.tensor_tensor(dst[H2:P, :], ta[H2:P, :], tb[H2:P, :], add)

        with tc.tile_pool(name="wqbp", bufs=3) as wqbp, \
             tc.tile_pool(name="rtmp", bufs=3) as rtmp, \
             tc.tile_pool(name="ps_qb", bufs=3, space="PSUM") as psqb:
            for h in range(HH):
                w_h = wqbp.tile([P, QRK, P], F32R, tag="wqb")
                nc.sync.dma_start(w_h[:], wqb[h])
                ps = psqb.tile([P, TN, 512], F32, tag="psqb")
                for tn in range(TN):
                    ts = slice(tn * 512, (tn + 1) * 512)
                    for m in range(QRK):
                        nc.tensor.matmul(ps[:, tn, :], w_h[:, m, :],
                                         qnT[:, m, ts],
                                         start=(m == 0), stop=(m == QRK - 1))
                rope_gate(qfT[:, h, :], ps[:].rearrange("p a b -> p (a b)"),
                          slice(0, T), C1q, S1q)

        kvp = top.enter_context(tc.tile_pool(name="kvp", bufs=1))
        kfT = kvp.tile([P, T], F32R)           # 4KB/p
        v_sb = kvp.tile([P, KT, P], F32R)      # 4KB/p
        with tc.tile_pool(name="wkvbp", bufs=1) as wkvbp, \
             tc.tile_pool(name="rtmp", bufs=4) as rtmp, \
             tc.tile_pool(name="ps_kv", bufs=1, space="PSUM") as pskv, \
             tc.tile_pool(name="ps_kvv", bufs=3, space="PSUM") as pskvv:
            wb = wkvbp.tile([P, 2, KVK, P], F32R)
            nc.sync.dma_start(wb[:], wkvb.rearrange("j p k c -> p j k c"))
            psk = pskv.tile([P, TN, 512], F32, tag="pskv")
            for tn in range(TN):
                ts = slice(tn * 512, (tn + 1) * 512)
                for m in range(KVK):
                    nc.tensor.matmul(psk[:, tn, :], wb[:, 0, m, :],
                                     kvnT[:, m, ts],
                                     start=(m == 0), stop=(m == KVK - 1))
            rope_gate(kfT[:, :], psk[:].rearrange("p a b -> p (a b)"),
                      slice(0, T), C1, S1)
            for vt in range(KT):
                vs = slice(vt * P, (vt + 1) * P)
                ps = pskvv.tile([P, P], F32, tag="pskv_v")
                for m in range(KVK):
                    nc.tensor.matmul(ps[:], kvnT[:, m, vs], wb[:, 1, m, :],
                                     start=(m == 0), stop=(m == KVK - 1))
                nc.scalar.copy(v_sb[:, vt, :], ps[:])

        if dbg:
            nc.sync.dma_start(dbg_qfT[:], qfT[:].bitcast(F32))
            nc.sync.dma_start(dbg_kfT[:], kfT[:].bitcast(F32))
            nc.sync.dma_start(dbg_v[:], v_sb[:].bitcast(F32))
        # ---------- attention (fully transposed softmax) ----------
        outp = top.enter_context(tc.tile_pool(name="outp", bufs=1))
        outT = outp.tile([P, HH, T], F32R)     # 32KB/p
        wop = top.enter_context(tc.tile_pool(name="wop", bufs=3))
        w_nt_pre = {}

        def load_wo(nt):
            halves = []
            for hf in range(2):
                t = wop.tile([P, 4, 512], F32R, tag="wo",
                             name=f"wo_{nt}_{hf}")
                nc.sync.dma_start(
                    t[:], wo[nt, hf * 4:(hf + 1) * 4].rearrange(
                        "h p n -> p h n"))
                halves.append(t)
            return halves

        w_nt_pre[0] = load_wo(0)
        with tc.tile_pool(name="expp", bufs=3) as expp, \
             tc.tile_pool(name="atmp", bufs=3) as atmp, \
             tc.tile_pool(name="ps_s", bufs=3, space="PSUM") as ps_s, \
             tc.tile_pool(name="ps_o", bufs=2, space="PSUM") as ps_o, \
             tc.tile_pool(name="ps_r", bufs=2, space="PSUM") as ps_r:
            for qb in range(NQB):
                qs = slice(qb * QB, (qb + 1) * QB)
                npair = qb + 1            # causal: key-chunk pairs 0..qb
                for h in range(HH):
                    po = ps_o.tile([P, QB], F32, tag="po")
                    pr = ps_r.tile([1, QB], F32, tag="pr")
                    for pc in range(npair):
                        pss = ps_s.tile([P, 2, QB], F32, tag="pss")
                        for j in range(2):
                            kc = 2 * pc + j
                            nc.tensor.matmul(
                                pss[:, j, :], kfT[:, kc * P:(kc + 1) * P],
                                qfT[:, h, qs], start=True,
                                stop=(not use_pad))
                            if use_pad:
                                nc.tensor.matmul(
                                    pss[:, j, :], padr[:, kc * P:(kc + 1) * P],
                                    ones_row2[:, :QB], start=False, stop=True)
                        if pc == npair - 1:   # diagonal pair: causal triangle
                            nc.vector.tensor_tensor(pss[:], pss[:], dm_sb[:],
                                                    add)
                        es = expp.tile([P, 2, QB], F32R, tag="es")
                        nc.scalar.activation(es[:], pss[:], AF.Exp, bias=0.0,
                                             scale=SCALE)
                        for j in range(2):
                            kc = 2 * pc + j
                            nc.tensor.matmul(po[:], v_sb[:, kc, :],
                                             es[:, j, :], start=(kc == 0),
                                             stop=(kc == 2 * npair - 1))
                            nc.tensor.matmul(pr[:], ones_col, es[:, j, :],
                                             start=(kc == 0),
                                             stop=(kc == 2 * npair - 1))
                    r1r = atmp.tile([1, QB], F32, tag="r1r")
                    nc.vector.reciprocal(r1r[:], pr[:])
                    rb = atmp.tile([P, QB], F32, tag="rb")
                    nc.gpsimd.partition_broadcast(rb[:], r1r[:])
                    nc.vector.tensor_tensor(outT[:, h, qs], po[:], rb[:], mul)

        if dbg:
            nc.sync.dma_start(dbg_outT[:], outT[:].bitcast(F32))
        # ---------- o_proj ----------
        with tc.tile_pool(name="osb", bufs=2) as osb, \
             tc.tile_pool(name="ps_w", bufs=4, space="PSUM") as psw:
            for nt in range(4):
                w_nt = w_nt_pre[nt] if nt in w_nt_pre else load_wo(nt)
                ns = slice(nt * 512, (nt + 1) * 512)
                for half in range(4):
                    ot = osb.tile([P, 2, 512], F32, tag="ot")
                    for qq in range(2):
                        qt = half * 2 + qq
                        qs = slice(qt * P, (qt + 1) * P)
                        ps = psw.tile([P, 512], F32, tag="psw")
                        for h in range(HH):
                            nc.tensor.matmul(ps[:], outT[:, h, qs],
                                             w_nt[h // 4][:, h % 4, :],
                                             start=(h == 0),
                                             stop=(h == HH - 1))
                        nc.vector.tensor_copy(ot[:, qq, :], ps[:])
                    nc.sync.dma_start(
                        o_part[half * 256:(half + 1) * 256, ns]
                        .rearrange("(q p) c -> p q c", p=P), ot[:])

    nc.finalize()
    return nc


def _prep_core_inputs(inputs):
    """Shard + lay out the full inputs for the 8 cores.

    Returns a list of 8 dicts keyed by the dram tensor names above.
    """
    f32 = np.float32
    hs = np.ascontiguousarray(np.asarray(inputs["hidden_states"], f32))
    w_qa = np.asarray(inputs["w_qa"], f32)
    b_qa = np.asarray(inputs["b_qa"], f32)
    w_qb = np.asarray(inputs["w_qb"], f32)
    b_qb = np.asarray(inputs["b_qb"], f32)
    w_kva = np.asarray(inputs["w_kva"], f32)
    b_kva = np.asarray(inputs["b_kva"], f32)
    w_kvb = np.asarray(inputs["w_kvb"], f32)
    b_kvb = np.asarray(inputs["b_kvb"], f32)
    qn_w = np.asarray(inputs["qn_w"], f32)
    kvn_w = np.asarray(inputs["kvn_w"], f32)
    w_o = np.asarray(inputs["w_o"], f32)
    att_mask = np.asarray(inputs["attention_mask"])
    assert not b_qa.any() and not b_qb.any() and not b_kva.any() \
        and not b_kvb.any(), "nonzero projection biases not supported"

    # fold RMS-norm weights into the up-projections
    w_qb_f = qn_w[:, None] * w_qb          # [QR, H*HD]
    w_kvb_f = kvn_w[:, None] * w_kvb       # [KVR, 2*HD]

    wqa_pre = np.ascontiguousarray(
        w_qa.reshape(DK, P, QRK, P).transpose(2, 1, 0, 3))
    wkva_pre = np.ascontiguousarray(
        w_kva.reshape(DK, P, KVK, P).transpose(2, 1, 0, 3))
    wkvb_pre = np.ascontiguousarray(
        w_kvb_f.reshape(KVK, P, 2, HD).transpose(2, 1, 0, 3))

    trig_full = np.stack([
        np.asarray(inputs["cos_g"], f32),
        np.asarray(inputs["cos_l"], f32),
        np.asarray(inputs["sin_g"], f32),
        np.asarray(inputs["sin_l"], f32),
    ])  # [4, B, T, H2]
    gates_np = np.array(
        [[np.float32(inputs["nope_logit"]),
          np.float32(inputs["rope_logit"])]], f32)

    # causal triangle masks for the two diagonal key chunks, [k, q] layout
    # chunk j covers keys qb*QB + j*P ... ; queries qb*QB ... qb*QB+QB
    i = np.arange(P)[:, None]
    j = np.arange(QB)[None, :]
    dm = np.zeros((P, 2, QB), f32)
    dm[:, 0, :] = np.where(i > j, NEG, 0.0)          # keys at block offset 0
    dm[:, 1, :] = np.where(i + P > j, NEG, 0.0)      # keys at offset 128

    in_maps = []
    for c in range(NCORES):
        b, hh = c // 2, c % 2
        hidT = np.ascontiguousarray(hs[b].T.reshape(DK, P, T))
        w_qb_h = w_qb_f[:, hh * HH * HD:(hh + 1) * HH * HD]
        wqb_pre = np.ascontiguousarray(
            w_qb_h.reshape(QRK, P, HH, HD).transpose(2, 1, 0, 3))
        w_o_h = w_o[hh * HH * HD:(hh + 1) * HH * HD, :]
        wo_pre = np.ascontiguousarray(
            w_o_h.reshape(HH, P, 4, 512).transpose(2, 0, 1, 3))
        trig_b = np.ascontiguousarray(trig_full[:, b].transpose(0, 2, 1))
        padb_np = np.ascontiguousarray(
            np.where(att_mask[b] == 0, NEG, 0.0).astype(f32)[None, :])
        in_maps.append({
            "hid": hidT, "wqa": wqa_pre, "wkva": wkva_pre,
            "wqb": wqb_pre, "wkvb": wkvb_pre, "wo": wo_pre,
            "trig": trig_b, "gates": gates_np, "padb": padb_np,
            "dmask": dm, "onesm": np.ones((P, QB), f32),
        })
    return in_maps


NCORES = 8


def kernel(**inputs):
    use_pad = bool((np.asarray(inputs["attention_mask"]) == 0).any())
    if use_pad not in _nc_cache:
        _nc_cache[use_pad] = build_kernel(use_pad=use_pad)
    nc = _nc_cache[use_pad]

    from concourse.bass_utils import run_bass_kernel_spmd
    in_maps = _prep_core_inputs(inputs)
    res = run_bass_kernel_spmd(nc, in_maps, core_ids=list(range(NCORES)))
    out = np.empty((B, T, D), np.float32)
    for b in range(B):
        out[b] = res.results[2 * b]["o_part"] + res.results[2 * b + 1]["o_part"]
    return out



# revision 40
# speedup vs baseline: 1.1946x; 1.1946x over previous
"""MLA/MQA attention block (nn_Attention_33406255628587) on 8 Trainium2 cores.

Sharding: data-parallel over batch (4) x tensor-parallel over query heads
(16 -> 2 groups of 8).  Core c handles batch c//2, head group c%2.  All 8
cores run one SPMD program; the host sums the two o_proj partials of each
batch pair (o_proj contracts over heads, so head-split partials add).

v2 changes vs the first working kernel (375us cost-model estimate):
  - bf16 operands everywhere precision allows (rel err ~3e-3 measured on
    host): halves DMA bytes, halves SBUF, enables the DVE 2x 16-bit mode
    for the RoPE/mask elementwise work.  PSUM accumulation stays fp32.
  - kv-down runs BEFORE q-down so the kv up-projection + k RoPE + v copy
    hide inside the q-down matmul stream and attention never waits on kv.
  - RoPE lowered from 5 full-width DVE passes to 3 bf16 passes: the
    partition half-swap is done by a cheap SBUF->SBUF DMA and the lower
    half of the sin table is pre-negated so both halves combine with a
    single subtract.
  - attention: all score matmuls of a (qb, h) issue back-to-back before
    the exp-dependent AV/denominator matmuls (no PE head-of-line stall on
    the Act exp); for the shallow qb=0/1 rounds the scores+exp of ALL
    heads batch ahead of the AV chains; causal mask applied
    multiplicatively (0/1) AFTER exp so it is off the scores critical
    path.  (NB the AV accumulation and its denominator must live in
    SEPARATE PSUM banks - sharing one bank corrupts both groups.)
  - o_proj interleaved into the attention phase per 128-query block (its
    matmuls fill PE idle while Act runs exp); bf16 partials out.
  - weights prefetched early and spread across the three DMA-capable
    queues (SP/Act/Pool), first-needed chunks ahead of bulk prefetch.
"""

import sys

sys.path.insert(0, "/opt/trn_rl_repo")

import numpy as np

B, T, D, H, HD = 4, 1024, 2048, 16, 128
QR, KVR = 1536, 512
EPS = 1e-6
NEG = -1000000000.0
SCALE = HD ** -0.5

P = 128
H2 = HD // 2      # 64
HH = H // 2       # 8 heads per core
QB = 256          # query block in attention
NQB = T // QB     # 4
KT = T // P       # 8 key chunks
DK = D // P       # 16
QRK = QR // P     # 12
KVK = KVR // P    # 4
TN = T // 512     # 2

_nc_cache = {}


def build_kernel(dbg=False, use_pad=False):
    import concourse.bacc as bacc
    import concourse.tile as tile
    from concourse import mybir
    from contextlib import ExitStack

    F32 = mybir.dt.float32
    F32R = mybir.dt.float32r
    BF = mybir.dt.bfloat16
    AF = mybir.ActivationFunctionType
    mul = mybir.AluOpType.mult
    add = mybir.AluOpType.add
    sub = mybir.AluOpType.subtract

    nc = bacc.Bacc("TRN2", target_bir_lowering=False, debug=False)

    # ---- DRAM I/O (host-prepared layouts, see _prep_core_inputs) ----
    hid = nc.dram_tensor("hid", [DK, P, T], BF, kind="ExternalInput")
    wqa = nc.dram_tensor("wqa", [QRK, P, DK, P], BF, kind="ExternalInput")
    wkva = nc.dram_tensor("wkva", [KVK, P, DK, P], BF, kind="ExternalInput")
    wqb = nc.dram_tensor("wqb", [HH, P, QRK, P], BF, kind="ExternalInput")
    wkvb = nc.dram_tensor("wkvb", [P, 2, KVK, P], BF, kind="ExternalInput")
    wo = nc.dram_tensor("wo", [4, 2, P, 4, 512], BF, kind="ExternalInput")
    trig = nc.dram_tensor("trig", [4, H2, T], BF, kind="ExternalInput")
    gates = nc.dram_tensor("gates", [1, 2], F32, kind="ExternalInput")
    padb = nc.dram_tensor("padb", [1, T], F32R, kind="ExternalInput")
    dmask = nc.dram_tensor("dmask", [P, 2, QB], BF, kind="ExternalInput")
    onesm = nc.dram_tensor("onesm", [P, QB], BF, kind="ExternalInput")
    o_part = nc.dram_tensor("o_part", [T, D], BF, kind="ExternalOutput")
    if dbg:
        dbg_qnT = nc.dram_tensor("dbg_qnT", [P, QRK, T], F32, kind="ExternalOutput")
        dbg_kvnT = nc.dram_tensor("dbg_kvnT", [P, KVK, T], F32, kind="ExternalOutput")
        dbg_qfT = nc.dram_tensor("dbg_qfT", [P, HH, T], F32, kind="ExternalOutput")
        dbg_kfT = nc.dram_tensor("dbg_kfT", [P, T], F32, kind="ExternalOutput")
        dbg_v = nc.dram_tensor("dbg_v", [P, KT, P], F32, kind="ExternalOutput")
        dbg_outT = nc.dram_tensor("dbg_outT", [P, HH, T], F32, kind="ExternalOutput")
        dbg_C1 = nc.dram_tensor("dbg_C1", [P, T], F32, kind="ExternalOutput")
        dbg_S1 = nc.dram_tensor("dbg_S1", [P, T], F32, kind="ExternalOutput")

    with tile.TileContext(nc, pool_alloc_mode="queue") as tc, ExitStack() as top:
        # ---------- persistent pools ----------
        consts = top.enter_context(tc.tile_pool(name="consts", bufs=1))
        latp = top.enter_context(tc.tile_pool(name="latp", bufs=1))

        qnT = latp.tile([P, QRK, T], BF)       # 24KB/p
        kvnT = latp.tile([P, KVK, T], BF)      # 8KB/p
        qfp = top.enter_context(tc.tile_pool(name="qfp", bufs=1))
        qfT = qfp.tile([P, HH, T], BF)         # 16KB/p
        kvp = top.enter_context(tc.tile_pool(name="kvp", bufs=1))
        kfT = kvp.tile([P, T], BF)             # 2KB/p
        v_sb = kvp.tile([P, KT, P], BF)        # 2KB/p
        outp = top.enter_context(tc.tile_pool(name="outp", bufs=1))
        outT = outp.tile([P, HH, T], BF)       # 16KB/p
        wop = top.enter_context(tc.tile_pool(name="wop", bufs=1))
        wo_sb = [[wop.tile([P, 4, 512], BF, name=f"wo_{nt}_{hf}")
                  for hf in range(2)] for nt in range(4)]

        # ---------- hidden^T tiles + early weight DMAs ----------
        hctx = ExitStack()
        hidp = hctx.enter_context(tc.tile_pool(name="hidp", bufs=1))
        normp = hctx.enter_context(tc.tile_pool(name="normp", bufs=1))
        wkvap = hctx.enter_context(tc.tile_pool(name="wkvap", bufs=3))
        hid_sb = [hidp.tile([P, T], BF, name=f"hid_{k}") for k in range(DK)]
        # kv-down weights first (kv-down runs first), interleaved with hid
        wkva_sb = []
        for m in range(KVK):
            w_m = wkvap.tile([P, DK, P], BF, tag="wkva", name=f"wkva_{m}")
            wkva_sb.append(w_m)
            if m < 2:
                nc.scalar.dma_start(hid_sb[m][:], hid[m])
            nc.sync.dma_start(w_m[:], wkva[m])
        for k in range(2, DK):
            nc.scalar.dma_start(hid_sb[k][:], hid[k])

        # small consts (after the critical first DMAs)
        padr = consts.tile([1, T], BF)
        nc.gpsimd.dma_start(padr[:], padb[:])
        dm_sb = consts.tile([P, 2, QB], BF)
        nc.gpsimd.dma_start(dm_sb[:], dmask[:])
        ones_mat = consts.tile([P, QB], BF)
        nc.gpsimd.dma_start(ones_mat[:], onesm[:])
        ones_col = ones_mat[:, 0:1]
        ones_row = ones_mat[0:1, 0:P]
        ones_row2 = ones_mat[0:1, :]
        C1 = consts.tile([P, T], BF)
        S1s = consts.tile([P, T], BF)          # sign-folded: lower half = -S
        C1q = consts.tile([P, T], BF)
        S1qs = consts.tile([P, T], BF)
        eps_sb = consts.tile([P, 1], F32)
        nc.vector.memset(eps_sb[:], EPS)

        # q-down weights stream in behind the kv weights
        wqap = hctx.enter_context(tc.tile_pool(name="wqap", bufs=3))

        # ---------- gates + blended RoPE tables ----------
        with tc.tile_pool(name="trigp", bufs=1) as trigp:
            g_sb = trigp.tile([1, 2], F32)
            nc.gpsimd.dma_start(g_sb[:], gates[:])
            tg = trigp.tile([H2, 4, T], BF)
            for j in range(4):
                nc.gpsimd.dma_start(tg[:, j, :], trig[j])
            s14 = trigp.tile([1, 4], F32)      # (a, g, 1-a, 1-g)
            nc.scalar.activation(s14[:, 0:2], g_sb[:], AF.Sigmoid)
            nc.scalar.activation(s14[:, 2:4], s14[:, 0:2], AF.Identity,
                                 bias=1.0, scale=-1.0)
            s4 = trigp.tile([H2, 4], F32)
            nc.gpsimd.partition_broadcast(s4[:], s14[:])
            a_c, g_c = s4[:, 0:1], s4[:, 1:2]
            ia_c, ig_c = s4[:, 2:3], s4[:, 3:4]
            nia = trigp.tile([H2, 1], F32)     # -(1-a)
            nc.scalar.activation(nia[:], ia_c, AF.Identity, scale=-1.0)

            cb = trigp.tile([H2, T], BF)
            sb2 = trigp.tile([H2, T], BF)
            # cos_blend = g*cos_g + (1-g)*cos_l   (tg: cg, cl, sg, sl)
            nc.vector.tensor_scalar(out=cb[:], in0=tg[:, 0, :], scalar1=g_c,
                                    scalar2=None, op0=mul)
            nc.vector.scalar_tensor_tensor(out=cb[:], in0=tg[:, 1, :], scalar=ig_c,
                                           in1=cb[:], op0=mul, op1=add)
            nc.vector.tensor_scalar(out=sb2[:], in0=tg[:, 2, :], scalar1=g_c,
                                    scalar2=None, op0=mul)
            nc.vector.scalar_tensor_tensor(out=sb2[:], in0=tg[:, 3, :], scalar=ig_c,
                                           in1=sb2[:], op0=mul, op1=add)
            # C1 = (1-a)*cos_blend + a  (both halves)
            # S1s = +(1-a)*sin_blend (upper) / -(1-a)*sin_blend (lower)
            for off in (0, H2):
                nc.vector.tensor_scalar(out=C1[off:off + H2, :], in0=cb[:],
                                        scalar1=ia_c, scalar2=a_c,
                                        op0=mul, op1=add)
            nc.vector.tensor_scalar(out=S1s[0:H2, :], in0=sb2[:],
                                    scalar1=ia_c, scalar2=None, op0=mul)
            nc.vector.tensor_scalar(out=S1s[H2:P, :], in0=sb2[:],
                                    scalar1=nia[:, 0:1], scalar2=None, op0=mul)

        # ---------- down-projections (transposed) + RMS-norm ----------
        # qnT stays UNSCALED (its 1/rms folds into C1q/S1qs); kvnT is
        # scaled in place (v needs it too).
        def down_proj(latT, nchunks, w_tiles, name, n_pre=0):
            with tc.tile_pool(name=f"sq_{name}", bufs=2) as sqp, \
                 tc.tile_pool(name=f"ps_{name}", bufs=3, space="PSUM") as psp, \
                 tc.tile_pool(name=f"pss_{name}", bufs=1, space="PSUM") as pssp:
                ss = pssp.tile([P, TN, 512], F32)

                def finish_m(m, ps):
                    nc.scalar.copy(latT[:, m, :],
                                   ps[:].rearrange("p a b -> p (a b)"))
                    sq = sqp.tile([P, T], BF, tag="sq", name=f"sq_{name}_{m}")
                    nc.scalar.square(sq[:], latT[:, m, :])
                    for tn in range(TN):
                        nc.tensor.matmul(
                            ss[:, tn, :], ones_mat[:, 0:P],
                            sq[:, tn * 512:(tn + 1) * 512],
                            start=(m == 0), stop=(m == nchunks - 1))

                # first n_pre chunks k-outer: consume hid chunks as they land
                if n_pre:
                    w_pre = [w_tiles(m) for m in range(n_pre)]
                    ps_pre = [psp.tile([P, TN, 512], F32, tag="ps",
                                       name=f"ps_{name}_p{m}")
                              for m in range(n_pre)]
                    for k in range(DK):
                        for m in range(n_pre):
                            for tn in range(TN):
                                ts = slice(tn * 512, (tn + 1) * 512)
                                nc.tensor.matmul(
                                    ps_pre[m][:, tn, :], w_pre[m][:, k, :],
                                    hid_sb[k][:, ts],
                                    start=(k == 0), stop=(k == DK - 1))
                    for m in range(n_pre):
                        finish_m(m, ps_pre[m])
                for m in range(n_pre, nchunks):
                    w_m = w_tiles(m)
                    ps = psp.tile([P, TN, 512], F32, tag="ps")
                    for tn in range(TN):
                        ts = slice(tn * 512, (tn + 1) * 512)
                        for k in range(DK):
                            nc.tensor.matmul(
                                ps[:, tn, :], w_m[:, k, :], hid_sb[k][:, ts],
                                start=(k == 0), stop=(k == DK - 1))
                    finish_m(m, ps)
                # rs = 1/sqrt(mean(sq)+eps), already partition-broadcast
                rsb = normp.tile([P, T], BF, name=f"rsb_{name}")
                for tn in range(TN):
                    nc.scalar.activation(
                        rsb[:, tn * 512:(tn + 1) * 512], ss[:, tn, :],
                        AF.Sqrt, bias=eps_sb[:], scale=1.0 / (nchunks * P))
                with nc.allow_low_precision(reason="1/rms fits bf16"):
                    nc.vector.reciprocal(rsb[:], rsb[:])
                return rsb

        # kv first: its up-proj + RoPE hide inside the q-down stream
        rsb_kva = down_proj(kvnT, KVK, lambda m: wkva_sb[m], "kva", n_pre=2)
        for m in range(KVK):
            nc.vector.tensor_tensor(kvnT[:, m, :], kvnT[:, m, :],
                                    rsb_kva[:], mul)

        def wqa_tile(m, _cache={}):
            if m in _cache:
                return _cache.pop(m)
            w_m = wqap.tile([P, DK, P], BF, tag="wqa", name=f"wqa_{m}")
            nc.sync.dma_start(w_m[:], wqa[m])
            return w_m

        # first q-down weights BEFORE the bulk prefetch (DMA pipe order)
        wqa_pre = {m: wqa_tile(m) for m in range(3)}

        # prefetch up-proj weights while q-down computes (27KB/p bf16)
        wqbp = hctx.enter_context(tc.tile_pool(name="wqbp", bufs=1))
        wqb_sb = [wqbp.tile([P, QRK, P], BF, name=f"wqb_{h}") for h in range(HH)]
        wkvb_sb = wqbp.tile([P, 2, KVK, P], BF)
        nc.gpsimd.dma_start(wkvb_sb[:], wkvb[:])
        for h in range(HH):
            (nc.scalar if h % 2 else nc.gpsimd).dma_start(
                wqb_sb[h][:], wqb[h])

        rsb_qa = down_proj(
            qnT, QRK,
            lambda m: wqa_pre.pop(m) if m in wqa_pre else wqa_tile(m),
            "qa")
        nc.vector.tensor_tensor(C1q[:], C1[:], rsb_qa[:], mul)
        nc.vector.tensor_tensor(S1qs[:], S1s[:], rsb_qa[:], mul)

        if dbg:
            nc.gpsimd.dma_start(dbg_qnT[:], qnT[:])
            nc.gpsimd.dma_start(dbg_kvnT[:], kvnT[:])
            nc.gpsimd.dma_start(dbg_C1[:], C1[:])
            nc.gpsimd.dma_start(dbg_S1[:], S1s[:])

        # ---------- up-projections + RoPE/gating ----------
        rtctx = ExitStack()
        rtmp = rtctx.enter_context(tc.tile_pool(name="rtmp", bufs=1))

        def rope_gate(dst, ps, Ct, Sts):
            # dst = s*Ct - swap64(s)*Sts  with Sts lower half pre-negated.
            W = ps.shape[-1]
            s_sb = rtmp.tile([P, T], BF, tag="rs", name="rope_s")[:, :W]
            nc.scalar.copy(s_sb[:], ps[:])
            psw = rtmp.tile([P, T], BF, tag="rw", name="rope_w")[:, :W]
            nc.sync.dma_start(psw[0:H2, :], s_sb[H2:P, :])
            nc.sync.dma_start(psw[H2:P, :], s_sb[0:H2, :])
            ta = rtmp.tile([P, T], BF, tag="ra", name="rope_a")[:, :W]
            nc.vector.tensor_tensor(ta[:], s_sb[:], Ct[:, :W], mul)
            nc.vector.tensor_tensor(psw[:], psw[:], Sts[:, :W], mul)
            nc.vector.tensor_tensor(dst[:], ta[:], psw[:], sub)

        # kv up-proj + k RoPE + v copy (eagerly, overlaps q-down tail)
        with tc.tile_pool(name="ps_kv", bufs=1, space="PSUM") as pskv, \
             tc.tile_pool(name="ps_kvv", bufs=2, space="PSUM") as pskvv:
            psk = pskv.tile([P, TN, 512], F32, tag="pskv")
            for tn in range(TN):
                ts = slice(tn * 512, (tn + 1) * 512)
                for m in range(KVK):
                    nc.tensor.matmul(psk[:, tn, :], wkvb_sb[:, 0, m, :],
                                     kvnT[:, m, ts],
                                     start=(m == 0), stop=(m == KVK - 1))
            rope_gate(kfT[:, :], psk[:].rearrange("p a b -> p (a b)"), C1, S1s)
            for vt in range(KT):
                vs = slice(vt * P, (vt + 1) * P)
                ps = pskvv.tile([P, P], F32, tag="pskv_v")
                for m in range(KVK):
                    nc.tensor.matmul(ps[:], kvnT[:, m, vs], wkvb_sb[:, 1, m, :],
                                     start=(m == 0), stop=(m == KVK - 1))
                nc.scalar.copy(v_sb[:, vt, :], ps[:])

        with tc.tile_pool(name="ps_qb", bufs=3, space="PSUM") as psqb:
            for h in range(HH):
                ps = psqb.tile([P, TN, 512], F32, tag="psqb")
                for tn in range(TN):
                    ts = slice(tn * 512, (tn + 1) * 512)
                    for m in range(QRK):
                        nc.tensor.matmul(ps[:, tn, :], wqb_sb[h][:, m, :],
                                         qnT[:, m, ts],
                                         start=(m == 0), stop=(m == QRK - 1))
                rope_gate(qfT[:, h, :], ps[:].rearrange("p a b -> p (a b)"),
                          C1q, S1qs)

        rtctx.close()
        hctx.close()

        if dbg:
            nc.gpsimd.dma_start(dbg_qfT[:], qfT[:])
            nc.gpsimd.dma_start(dbg_kfT[:], kfT[:])
            nc.gpsimd.dma_start(dbg_v[:], v_sb[:])
        # ---------- attention + interleaved o_proj ----------
        for nt in range(4):
            for hf in range(2):
                (nc.scalar if (nt + hf) % 2 else nc.gpsimd).dma_start(
                    wo_sb[nt][hf][:], wo[nt, hf])

        with tc.tile_pool(name="expp", bufs=17) as expp, \
             tc.tile_pool(name="atmp", bufs=4) as atmp, \
             tc.tile_pool(name="ps_s", bufs=3, space="PSUM") as ps_s, \
             tc.tile_pool(name="ps_o", bufs=2, space="PSUM") as ps_o, \
             tc.tile_pool(name="ps_r", bufs=1, space="PSUM") as ps_r, \
             tc.tile_pool(name="ps_w", bufs=2, space="PSUM") as psw_pool:

            def scores(qs, h, pc, npair):
                pss = ps_s.tile([P, 2, QB], F32, tag="pss", name="pss")
                for j in range(2):
                    kc = 2 * pc + j
                    nc.tensor.matmul(
                        pss[:, j, :], kfT[:, kc * P:(kc + 1) * P],
                        qfT[:, h, qs], start=True, stop=(not use_pad))
                    if use_pad:
                        nc.tensor.matmul(
                            pss[:, j, :], padr[:, kc * P:(kc + 1) * P],
                            ones_row2[:, :QB], start=False, stop=True)
                es = expp.tile([P, 2, QB], BF, tag="es", name="es")
                nc.scalar.activation(es[:], pss[:], AF.Exp, bias=0.0,
                                     scale=SCALE)
                if pc == npair - 1:       # diagonal pair: causal 0/1 mask
                    nc.vector.tensor_tensor(es[:], es[:], dm_sb[:], mul)
                return es

            def av_norm(qs, h, es_t, npair):
                po2 = ps_o.tile([P, QB], F32, tag="po", name="po")
                pr = ps_r.tile([1, QB], F32, tag="pr", name="pr")
                for pc in range(npair):
                    for j in range(2):
                        kc = 2 * pc + j
                        nc.tensor.matmul(po2[:], v_sb[:, kc, :],
                                         es_t[pc][:, j, :], start=(kc == 0),
                                         stop=(kc == 2 * npair - 1))
                        nc.tensor.matmul(pr[:], ones_col,
                                         es_t[pc][:, j, :], start=(kc == 0),
                                         stop=(kc == 2 * npair - 1))
                r1r = atmp.tile([1, QB], F32, tag="r1r", name="r1r")
                nc.vector.reciprocal(r1r[:], pr[:])
                rb = atmp.tile([P, QB], F32, tag="rb", name="rb")
                nc.gpsimd.partition_broadcast(rb[:], r1r[:])
                nc.vector.tensor_tensor(outT[:, h, qs], po2[:], rb[:], mul)

            for qb in range(NQB):
                qs = slice(qb * QB, (qb + 1) * QB)
                npair = qb + 1            # causal: key-chunk pairs 0..qb
                if npair <= 2:
                    # shallow rounds: batch ALL scores+exp across heads so
                    # the PE never stalls on Act inside a head
                    es_all = [[scores(qs, h, pc, npair)
                               for pc in range(npair)] for h in range(HH)]
                    for h in range(HH):
                        av_norm(qs, h, es_all[h], npair)
                else:
                    for h in range(HH):
                        es_t = [scores(qs, h, pc, npair)
                                for pc in range(npair)]
                        av_norm(qs, h, es_t, npair)
                # o_proj for the two finished 128-query blocks
                for qq in range(2):
                    qt = 2 * qb + qq
                    qsl = slice(qt * P, (qt + 1) * P)
                    ot = atmp.tile([P, 4, 512], BF, tag="ot")
                    for nt in range(4):
                        psw = psw_pool.tile([P, 512], F32, tag="psw")
                        for h in range(HH):
                            nc.tensor.matmul(psw[:], outT[:, h, qsl],
                                             wo_sb[nt][h // 4][:, h % 4, :],
                                             start=(h == 0),
                                             stop=(h == HH - 1))
                        nc.scalar.copy(ot[:, nt, :], psw[:])
                    nc.sync.dma_start(
                        o_part[qt * P:(qt + 1) * P, :], ot[:])

        if dbg:
            nc.gpsimd.dma_start(dbg_outT[:], outT[:])

    nc.finalize()
    return nc


def _prep_core_inputs(inputs):
    """Shard + lay out the full inputs for the 8 cores.

    Returns a list of 8 dicts keyed by the dram tensor names above.
    """
    import ml_dtypes
    f32 = np.float32
    bf16 = ml_dtypes.bfloat16
    hs = np.ascontiguousarray(np.asarray(inputs["hidden_states"], f32))
    w_qa = np.asarray(inputs["w_qa"], f32)
    b_qa = np.asarray(inputs["b_qa"], f32)
    w_qb = np.asarray(inputs["w_qb"], f32)
    b_qb = np.asarray(inputs["b_qb"], f32)
    w_kva = np.asarray(inputs["w_kva"], f32)
    b_kva = np.asarray(inputs["b_kva"], f32)
    w_kvb = np.asarray(inputs["w_kvb"], f32)
    b_kvb = np.asarray(inputs["b_kvb"], f32)
    qn_w = np.asarray(inputs["qn_w"], f32)
    kvn_w = np.asarray(inputs["kvn_w"], f32)
    w_o = np.asarray(inputs["w_o"], f32)
    att_mask = np.asarray(inputs["attention_mask"])
    assert not b_qa.any() and not b_qb.any() and not b_kva.any() \
        and not b_kvb.any(), "nonzero projection biases not supported"

    # fold RMS-norm weights into the up-projections
    w_qb_f = qn_w[:, None] * w_qb          # [QR, H*HD]
    w_kvb_f = kvn_w[:, None] * w_kvb       # [KVR, 2*HD]

    wqa_pre = np.ascontiguousarray(
        w_qa.reshape(DK, P, QRK, P).transpose(2, 1, 0, 3)).astype(bf16)
    wkva_pre = np.ascontiguousarray(
        w_kva.reshape(DK, P, KVK, P).transpose(2, 1, 0, 3)).astype(bf16)
    wkvb_pre = np.ascontiguousarray(
        w_kvb_f.reshape(KVK, P, 2, HD).transpose(1, 2, 0, 3)).astype(bf16)

    trig_full = np.stack([
        np.asarray(inputs["cos_g"], f32),
        np.asarray(inputs["cos_l"], f32),
        np.asarray(inputs["sin_g"], f32),
        np.asarray(inputs["sin_l"], f32),
    ])  # [4, B, T, H2]
    gates_np = np.array(
        [[np.float32(inputs["nope_logit"]),
          np.float32(inputs["rope_logit"])]], f32)

    # causal 0/1 keep-masks for the two diagonal key chunks, [k, q] layout
    i = np.arange(P)[:, None]
    j = np.arange(QB)[None, :]
    dm = np.zeros((P, 2, QB), f32)
    dm[:, 0, :] = (i <= j)
    dm[:, 1, :] = (i + P <= j)
    dm = dm.astype(bf16)

    in_maps = []
    for c in range(NCORES):
        b, hh = c // 2, c % 2
        hidT = np.ascontiguousarray(
            hs[b].T.reshape(DK, P, T).astype(bf16))
        w_qb_h = w_qb_f[:, hh * HH * HD:(hh + 1) * HH * HD]
        wqb_pre = np.ascontiguousarray(
            w_qb_h.reshape(QRK, P, HH, HD).transpose(2, 1, 0, 3)).astype(bf16)
        w_o_h = w_o[hh * HH * HD:(hh + 1) * HH * HD, :]
        wo_pre = np.ascontiguousarray(
            w_o_h.reshape(2, 4, P, 4, 512).transpose(3, 0, 2, 1, 4)).astype(bf16)
        trig_b = np.ascontiguousarray(
            trig_full[:, b].transpose(0, 2, 1)).astype(bf16)
        padb_np = np.ascontiguousarray(
            np.where(att_mask[b] == 0, NEG, 0.0).astype(f32)[None, :])
        in_maps.append({
            "hid": hidT, "wqa": wqa_pre, "wkva": wkva_pre,
            "wqb": wqb_pre, "wkvb": wkvb_pre, "wo": wo_pre,
            "trig": trig_b, "gates": gates_np, "padb": padb_np,
            "dmask": dm, "onesm": np.ones((P, QB), bf16),
        })
    return in_maps


NCORES = 8


def kernel(**inputs):
    use_pad = bool((np.asarray(inputs["attention_mask"]) == 0).any())
    if use_pad not in _nc_cache:
        _nc_cache[use_pad] = build_kernel(use_pad=use_pad)
    nc = _nc_cache[use_pad]

    from concourse.bass_utils import run_bass_kernel_spmd
    in_maps = _prep_core_inputs(inputs)
    res = run_bass_kernel_spmd(nc, in_maps, core_ids=list(range(NCORES)))
    out = np.empty((B, T, D), np.float32)
    for b in range(B):
        out[b] = (res.results[2 * b]["o_part"].astype(np.float32)
                  + res.results[2 * b + 1]["o_part"].astype(np.float32))
    return out


# revision 64
# speedup vs baseline: 1.2097x; 1.0126x over previous
"""MLA/MQA attention block (nn_Attention_33406255628587) on 8 Trainium2 cores.

Sharding: data-parallel over batch (4) x tensor-parallel over query heads
(16 -> 2 groups of 8).  Core c handles batch c//2, head group c%2.  All 8
cores run one SPMD program; the host sums the two o_proj partials of each
batch pair (o_proj contracts over heads, so head-split partials add).

v2 changes vs the first working kernel (375us cost-model estimate):
  - bf16 operands everywhere precision allows (rel err ~3e-3 measured on
    host): halves DMA bytes, halves SBUF, enables the DVE 2x 16-bit mode
    for the RoPE/mask elementwise work.  PSUM accumulation stays fp32.
  - kv-down runs BEFORE q-down so the kv up-projection + k RoPE + v copy
    hide inside the q-down matmul stream and attention never waits on kv.
  - RoPE lowered from 5 full-width DVE passes to 3 bf16 passes: the
    partition half-swap is done by a cheap SBUF->SBUF DMA and the lower
    half of the sin table is pre-negated so both halves combine with a
    single subtract.
  - attention: all score matmuls of a (qb, h) issue back-to-back before
    the exp-dependent AV/denominator matmuls (no PE head-of-line stall on
    the Act exp); for the shallow qb=0/1 rounds the scores+exp of ALL
    heads batch ahead of the AV chains; causal mask applied
    multiplicatively (0/1) AFTER exp so it is off the scores critical
    path.  (NB the AV accumulation and its denominator must live in
    SEPARATE PSUM banks - sharing one bank corrupts both groups.)
  - o_proj interleaved into the attention phase per 128-query block (its
    matmuls fill PE idle while Act runs exp); bf16 partials out.
  - weights prefetched early and spread across the three DMA-capable
    queues (SP/Act/Pool), first-needed chunks ahead of bulk prefetch.
"""

import sys

sys.path.insert(0, "/opt/trn_rl_repo")

import numpy as np

B, T, D, H, HD = 4, 1024, 2048, 16, 128
QR, KVR = 1536, 512
EPS = 1e-6
NEG = -1000000000.0
SCALE = HD ** -0.5

P = 128
H2 = HD // 2      # 64
HH = H // 2       # 8 heads per core
QB = 256          # query block in attention
NQB = T // QB     # 4
KT = T // P       # 8 key chunks
DK = D // P       # 16
QRK = QR // P     # 12
KVK = KVR // P    # 4
TN = T // 512     # 2

_nc_cache = {}


def build_kernel(dbg=False, use_pad=False):
    import concourse.bacc as bacc
    import concourse.tile as tile
    from concourse import mybir
    from contextlib import ExitStack

    F32 = mybir.dt.float32
    F32R = mybir.dt.float32r
    BF = mybir.dt.bfloat16
    AF = mybir.ActivationFunctionType
    mul = mybir.AluOpType.mult
    add = mybir.AluOpType.add
    sub = mybir.AluOpType.subtract

    nc = bacc.Bacc("TRN2", target_bir_lowering=False, debug=False)

    # ---- DRAM I/O (host-prepared layouts, see _prep_core_inputs) ----
    hid = nc.dram_tensor("hid", [DK, P, T], BF, kind="ExternalInput")
    wqa = nc.dram_tensor("wqa", [QRK, P, DK, P], BF, kind="ExternalInput")
    wkva = nc.dram_tensor("wkva", [KVK, P, DK, P], BF, kind="ExternalInput")
    wqb = nc.dram_tensor("wqb", [HH, P, QRK, P], BF, kind="ExternalInput")
    wkvb = nc.dram_tensor("wkvb", [P, 2, KVK, P], BF, kind="ExternalInput")
    wo = nc.dram_tensor("wo", [4, 2, P, 4, 512], BF, kind="ExternalInput")
    trig = nc.dram_tensor("trig", [4, H2, T], BF, kind="ExternalInput")
    gates = nc.dram_tensor("gates", [1, 2], F32, kind="ExternalInput")
    padb = nc.dram_tensor("padb", [1, T], F32R, kind="ExternalInput")
    dmask = nc.dram_tensor("dmask", [P, 2, QB], BF, kind="ExternalInput")
    onesm = nc.dram_tensor("onesm", [P, QB], BF, kind="ExternalInput")
    o_part = nc.dram_tensor("o_part", [T, D], BF, kind="ExternalOutput")
    if dbg:
        dbg_qnT = nc.dram_tensor("dbg_qnT", [P, QRK, T], F32, kind="ExternalOutput")
        dbg_kvnT = nc.dram_tensor("dbg_kvnT", [P, KVK, T], F32, kind="ExternalOutput")
        dbg_qfT = nc.dram_tensor("dbg_qfT", [P, HH, T], F32, kind="ExternalOutput")
        dbg_kfT = nc.dram_tensor("dbg_kfT", [P, T], F32, kind="ExternalOutput")
        dbg_v = nc.dram_tensor("dbg_v", [P, KT, P], F32, kind="ExternalOutput")
        dbg_outT = nc.dram_tensor("dbg_outT", [P, HH, T], F32, kind="ExternalOutput")
        dbg_C1 = nc.dram_tensor("dbg_C1", [P, T], F32, kind="ExternalOutput")
        dbg_S1 = nc.dram_tensor("dbg_S1", [P, T], F32, kind="ExternalOutput")

    with tile.TileContext(nc, pool_alloc_mode="queue") as tc, ExitStack() as top:
        # ---------- persistent pools ----------
        consts = top.enter_context(tc.tile_pool(name="consts", bufs=1))
        latp = top.enter_context(tc.tile_pool(name="latp", bufs=1))

        qnT = latp.tile([P, QRK, T], BF)       # 24KB/p
        kvnT = latp.tile([P, KVK, T], BF)      # 8KB/p
        qfp = top.enter_context(tc.tile_pool(name="qfp", bufs=1))
        qfT = qfp.tile([P, HH, T], BF)         # 16KB/p
        kvp = top.enter_context(tc.tile_pool(name="kvp", bufs=1))
        kfT = kvp.tile([P, T], BF)             # 2KB/p
        v_sb = kvp.tile([P, KT, P], BF)        # 2KB/p
        outp = top.enter_context(tc.tile_pool(name="outp", bufs=1))
        outT = outp.tile([P, HH, T], BF)       # 16KB/p
        wop = top.enter_context(tc.tile_pool(name="wop", bufs=1))
        wo_sb = [[wop.tile([P, 4, 512], BF, name=f"wo_{nt}_{hf}")
                  for hf in range(2)] for nt in range(4)]

        # ---------- hidden^T tiles + early weight DMAs ----------
        hctx = ExitStack()
        hidp = hctx.enter_context(tc.tile_pool(name="hidp", bufs=1))
        normp = hctx.enter_context(tc.tile_pool(name="normp", bufs=1))
        wkvap = hctx.enter_context(tc.tile_pool(name="wkvap", bufs=4))
        hid_sb = [hidp.tile([P, T], BF, name=f"hid_{k}") for k in range(DK)]
        # ALL kv-down weights first (kv-down runs first and must never wait
        # behind the bulk hid / q-weight stream), then hid
        wkva_sb = []
        for m in range(KVK):
            w_m = wkvap.tile([P, DK, P], BF, tag="wkva", name=f"wkva_{m}")
            wkva_sb.append(w_m)
            nc.sync.dma_start(w_m[:], wkva[m])
        for k in range(DK):
            nc.scalar.dma_start(hid_sb[k][:], hid[k])

        # small consts (after the critical first DMAs)
        padr = consts.tile([1, T], BF)
        nc.gpsimd.dma_start(padr[:], padb[:])
        dm_sb = consts.tile([P, 2, QB], BF)
        nc.gpsimd.dma_start(dm_sb[:], dmask[:])
        ones_mat = consts.tile([P, QB], BF)
        nc.gpsimd.dma_start(ones_mat[:], onesm[:])
        ones_col = ones_mat[:, 0:1]
        ones_row = ones_mat[0:1, 0:P]
        ones_row2 = ones_mat[0:1, :]
        C1 = consts.tile([P, T], BF)
        S1s = consts.tile([P, T], BF)          # sign-folded: lower half = -S
        C1q = consts.tile([P, T], BF)
        S1qs = consts.tile([P, T], BF)
        eps_sb = consts.tile([P, 1], F32)
        nc.vector.memset(eps_sb[:], EPS)

        # q-down weights stream in behind the kv weights
        wqap = hctx.enter_context(tc.tile_pool(name="wqap", bufs=3))

        # ---------- gates + blended RoPE tables ----------
        with tc.tile_pool(name="trigp", bufs=1) as trigp:
            g_sb = trigp.tile([1, 2], F32)
            nc.gpsimd.dma_start(g_sb[:], gates[:])
            tg = trigp.tile([H2, 4, T], BF)
            for j in range(4):
                nc.gpsimd.dma_start(tg[:, j, :], trig[j])
            s14 = trigp.tile([1, 4], F32)      # (a, g, 1-a, 1-g)
            nc.scalar.activation(s14[:, 0:2], g_sb[:], AF.Sigmoid)
            nc.scalar.activation(s14[:, 2:4], s14[:, 0:2], AF.Identity,
                                 bias=1.0, scale=-1.0)
            s4 = trigp.tile([H2, 4], F32)
            nc.gpsimd.partition_broadcast(s4[:], s14[:])
            a_c, g_c = s4[:, 0:1], s4[:, 1:2]
            ia_c, ig_c = s4[:, 2:3], s4[:, 3:4]
            nia = trigp.tile([H2, 1], F32)     # -(1-a)
            nc.scalar.activation(nia[:], ia_c, AF.Identity, scale=-1.0)

            cb = trigp.tile([H2, T], BF)
            sb2 = trigp.tile([H2, T], BF)
            # cos_blend = g*cos_g + (1-g)*cos_l   (tg: cg, cl, sg, sl)
            nc.vector.tensor_scalar(out=cb[:], in0=tg[:, 0, :], scalar1=g_c,
                                    scalar2=None, op0=mul)
            nc.vector.scalar_tensor_tensor(out=cb[:], in0=tg[:, 1, :], scalar=ig_c,
                                           in1=cb[:], op0=mul, op1=add)
            nc.vector.tensor_scalar(out=sb2[:], in0=tg[:, 2, :], scalar1=g_c,
                                    scalar2=None, op0=mul)
            nc.vector.scalar_tensor_tensor(out=sb2[:], in0=tg[:, 3, :], scalar=ig_c,
                                           in1=sb2[:], op0=mul, op1=add)
            # C1 = (1-a)*cos_blend + a  (both halves)
            # S1s = +(1-a)*sin_blend (upper) / -(1-a)*sin_blend (lower)
            for off in (0, H2):
                nc.vector.tensor_scalar(out=C1[off:off + H2, :], in0=cb[:],
                                        scalar1=ia_c, scalar2=a_c,
                                        op0=mul, op1=add)
            nc.vector.tensor_scalar(out=S1s[0:H2, :], in0=sb2[:],
                                    scalar1=ia_c, scalar2=None, op0=mul)
            nc.vector.tensor_scalar(out=S1s[H2:P, :], in0=sb2[:],
                                    scalar1=nia[:, 0:1], scalar2=None, op0=mul)

        # ---------- down-projections (transposed) + RMS-norm ----------
        # qnT stays UNSCALED (its 1/rms folds into C1q/S1qs); kvnT is
        # scaled in place (v needs it too).
        def down_proj(latT, nchunks, w_tiles, name, n_pre=0):
            with tc.tile_pool(name=f"sq_{name}", bufs=2) as sqp, \
                 tc.tile_pool(name=f"ps_{name}", bufs=3, space="PSUM") as psp, \
                 tc.tile_pool(name=f"pss_{name}", bufs=1, space="PSUM") as pssp:
                ss = pssp.tile([P, TN, 512], F32)

                def finish_m(m, ps):
                    nc.scalar.copy(latT[:, m, :],
                                   ps[:].rearrange("p a b -> p (a b)"))
                    sq = sqp.tile([P, T], BF, tag="sq", name=f"sq_{name}_{m}")
                    nc.scalar.square(sq[:], latT[:, m, :])
                    for tn in range(TN):
                        nc.tensor.matmul(
                            ss[:, tn, :], ones_mat[:, 0:P],
                            sq[:, tn * 512:(tn + 1) * 512],
                            start=(m == 0), stop=(m == nchunks - 1))

                # first n_pre chunks k-outer: consume hid chunks as they land
                if n_pre:
                    w_pre = [w_tiles(m) for m in range(n_pre)]
                    ps_pre = [psp.tile([P, TN, 512], F32, tag="ps",
                                       name=f"ps_{name}_p{m}")
                              for m in range(n_pre)]
                    for k in range(DK):
                        for m in range(n_pre):
                            for tn in range(TN):
                                ts = slice(tn * 512, (tn + 1) * 512)
                                nc.tensor.matmul(
                                    ps_pre[m][:, tn, :], w_pre[m][:, k, :],
                                    hid_sb[k][:, ts],
                                    start=(k == 0), stop=(k == DK - 1))
                    for m in range(n_pre):
                        finish_m(m, ps_pre[m])
                for m in range(n_pre, nchunks):
                    w_m = w_tiles(m)
                    ps = psp.tile([P, TN, 512], F32, tag="ps")
                    for tn in range(TN):
                        ts = slice(tn * 512, (tn + 1) * 512)
                        for k in range(DK):
                            nc.tensor.matmul(
                                ps[:, tn, :], w_m[:, k, :], hid_sb[k][:, ts],
                                start=(k == 0), stop=(k == DK - 1))
                    finish_m(m, ps)
                # rs = 1/sqrt(mean(sq)+eps), already partition-broadcast
                rsb = normp.tile([P, T], BF, name=f"rsb_{name}")
                for tn in range(TN):
                    nc.scalar.activation(
                        rsb[:, tn * 512:(tn + 1) * 512], ss[:, tn, :],
                        AF.Sqrt, bias=eps_sb[:], scale=1.0 / (nchunks * P))
                with nc.allow_low_precision(reason="1/rms fits bf16"):
                    nc.vector.reciprocal(rsb[:], rsb[:])
                return rsb

        # kv first: its up-proj + RoPE hide inside the q-down stream
        rsb_kva = down_proj(kvnT, KVK, lambda m: wkva_sb[m], "kva", n_pre=2)
        for m in range(KVK):
            nc.vector.tensor_tensor(kvnT[:, m, :], kvnT[:, m, :],
                                    rsb_kva[:], mul)

        def wqa_tile(m, _cache={}):
            if m in _cache:
                return _cache.pop(m)
            w_m = wqap.tile([P, DK, P], BF, tag="wqa", name=f"wqa_{m}")
            nc.sync.dma_start(w_m[:], wqa[m])
            return w_m

        # first q-down weights BEFORE the bulk prefetch (DMA pipe order)
        wqa_pre = {m: wqa_tile(m) for m in range(3)}

        # prefetch up-proj weights while q-down computes (27KB/p bf16)
        wqbp = hctx.enter_context(tc.tile_pool(name="wqbp", bufs=1))
        wqb_sb = [wqbp.tile([P, QRK, P], BF, name=f"wqb_{h}") for h in range(HH)]
        wkvb_sb = wqbp.tile([P, 2, KVK, P], BF)
        nc.gpsimd.dma_start(wkvb_sb[:], wkvb[:])
        for h in range(HH):
            (nc.scalar if h % 2 else nc.gpsimd).dma_start(
                wqb_sb[h][:], wqb[h])

        rsb_qa = down_proj(
            qnT, QRK,
            lambda m: wqa_pre.pop(m) if m in wqa_pre else wqa_tile(m),
            "qa")
        nc.vector.tensor_tensor(C1q[:], C1[:], rsb_qa[:], mul)
        nc.vector.tensor_tensor(S1qs[:], S1s[:], rsb_qa[:], mul)

        if dbg:
            nc.gpsimd.dma_start(dbg_qnT[:], qnT[:])
            nc.gpsimd.dma_start(dbg_kvnT[:], kvnT[:])
            nc.gpsimd.dma_start(dbg_C1[:], C1[:])
            nc.gpsimd.dma_start(dbg_S1[:], S1s[:])

        # ---------- up-projections + RoPE/gating ----------
        rtctx = ExitStack()
        rtmp = rtctx.enter_context(tc.tile_pool(name="rtmp", bufs=1))

        def rope_gate(dst, ps, Ct, Sts):
            # dst = s*Ct - swap64(s)*Sts  with Sts lower half pre-negated.
            W = ps.shape[-1]
            s_sb = rtmp.tile([P, T], BF, tag="rs", name="rope_s")[:, :W]
            nc.scalar.copy(s_sb[:], ps[:])
            psw = rtmp.tile([P, T], BF, tag="rw", name="rope_w")[:, :W]
            nc.sync.dma_start(psw[0:H2, :], s_sb[H2:P, :])
            nc.sync.dma_start(psw[H2:P, :], s_sb[0:H2, :])
            ta = rtmp.tile([P, T], BF, tag="ra", name="rope_a")[:, :W]
            nc.vector.tensor_tensor(ta[:], s_sb[:], Ct[:, :W], mul)
            nc.vector.tensor_tensor(psw[:], psw[:], Sts[:, :W], mul)
            nc.vector.tensor_tensor(dst[:], ta[:], psw[:], sub)

        # kv up-proj + k RoPE + v copy (eagerly, overlaps q-down tail)
        with tc.tile_pool(name="ps_kv", bufs=1, space="PSUM") as pskv, \
             tc.tile_pool(name="ps_kvv", bufs=2, space="PSUM") as pskvv:
            psk = pskv.tile([P, TN, 512], F32, tag="pskv")
            for tn in range(TN):
                ts = slice(tn * 512, (tn + 1) * 512)
                for m in range(KVK):
                    nc.tensor.matmul(psk[:, tn, :], wkvb_sb[:, 0, m, :],
                                     kvnT[:, m, ts],
                                     start=(m == 0), stop=(m == KVK - 1))
            rope_gate(kfT[:, :], psk[:].rearrange("p a b -> p (a b)"), C1, S1s)
            for vt in range(KT):
                vs = slice(vt * P, (vt + 1) * P)
                ps = pskvv.tile([P, P], F32, tag="pskv_v")
                for m in range(KVK):
                    nc.tensor.matmul(ps[:], kvnT[:, m, vs], wkvb_sb[:, 1, m, :],
                                     start=(m == 0), stop=(m == KVK - 1))
                nc.scalar.copy(v_sb[:, vt, :], ps[:])

        with tc.tile_pool(name="ps_qb", bufs=3, space="PSUM") as psqb:
            for h in range(HH):
                ps = psqb.tile([P, TN, 512], F32, tag="psqb")
                for tn in range(TN):
                    ts = slice(tn * 512, (tn + 1) * 512)
                    for m in range(QRK):
                        nc.tensor.matmul(ps[:, tn, :], wqb_sb[h][:, m, :],
                                         qnT[:, m, ts],
                                         start=(m == 0), stop=(m == QRK - 1))
                rope_gate(qfT[:, h, :], ps[:].rearrange("p a b -> p (a b)"),
                          C1q, S1qs)

        rtctx.close()
        hctx.close()

        if dbg:
            nc.gpsimd.dma_start(dbg_qfT[:], qfT[:])
            nc.gpsimd.dma_start(dbg_kfT[:], kfT[:])
            nc.gpsimd.dma_start(dbg_v[:], v_sb[:])
        # ---------- attention + interleaved o_proj ----------
        for nt in range(4):
            for hf in range(2):
                (nc.scalar if (nt + hf) % 2 else nc.gpsimd).dma_start(
                    wo_sb[nt][hf][:], wo[nt, hf])

        with tc.tile_pool(name="expp", bufs=17) as expp, \
             tc.tile_pool(name="atmp", bufs=4) as atmp, \
             tc.tile_pool(name="ps_s", bufs=3, space="PSUM") as ps_s, \
             tc.tile_pool(name="ps_o", bufs=2, space="PSUM") as ps_o, \
             tc.tile_pool(name="ps_r", bufs=1, space="PSUM") as ps_r, \
             tc.tile_pool(name="ps_w", bufs=2, space="PSUM") as psw_pool:

            def scores(qs, h, pc, npair):
                # Diagonal pair: chunk j=1 (keys at block offset 128) is
                # fully masked for queries 0..127 -> compute only the live
                # [128:QB] query half of it.
                diag = pc == npair - 1
                pss = ps_s.tile([P, 2, QB], F32, tag="pss", name="pss")
                es = expp.tile([P, 2, QB], BF, tag="es", name="es")
                for j in range(2):
                    kc = 2 * pc + j
                    q0 = P if (diag and j == 1) else 0
                    nc.tensor.matmul(
                        pss[:, j, q0:], kfT[:, kc * P:(kc + 1) * P],
                        qfT[:, h, qs.start + q0:qs.stop],
                        start=True, stop=(not use_pad))
                    if use_pad:
                        nc.tensor.matmul(
                            pss[:, j, q0:], padr[:, kc * P:(kc + 1) * P],
                            ones_row2[:, q0:QB], start=False, stop=True)
                if diag:
                    nc.scalar.activation(es[:, 0, :], pss[:, 0, :], AF.Exp,
                                         bias=0.0, scale=SCALE)
                    nc.scalar.activation(es[:, 1, P:], pss[:, 1, P:], AF.Exp,
                                         bias=0.0, scale=SCALE)
                    nc.vector.tensor_tensor(es[:, 0, :], es[:, 0, :],
                                            dm_sb[:, 0, :], mul)
                    nc.vector.tensor_tensor(es[:, 1, P:], es[:, 1, P:],
                                            dm_sb[:, 1, P:], mul)
                else:
                    nc.scalar.activation(es[:], pss[:], AF.Exp, bias=0.0,
                                         scale=SCALE)
                return es

            def av_norm(qs, h, es_t, npair):
                po2 = ps_o.tile([P, QB], F32, tag="po", name="po")
                pr = ps_r.tile([1, QB], F32, tag="pr", name="pr")
                for pc in range(npair):
                    diag = pc == npair - 1
                    for j in range(2):
                        kc = 2 * pc + j
                        q0 = P if (diag and j == 1) else 0
                        nc.tensor.matmul(po2[:, q0:], v_sb[:, kc, :],
                                         es_t[pc][:, j, q0:],
                                         start=(kc == 0),
                                         stop=(kc == 2 * npair - 1),
                                         skip_group_check=True)
                        nc.tensor.matmul(pr[:, q0:], ones_col,
                                         es_t[pc][:, j, q0:],
                                         start=(kc == 0),
                                         stop=(kc == 2 * npair - 1),
                                         skip_group_check=True)
                r1r = atmp.tile([1, QB], F32, tag="r1r", name="r1r")
                nc.vector.reciprocal(r1r[:], pr[:])
                rb = atmp.tile([P, QB], F32, tag="rb", name="rb")
                nc.gpsimd.partition_broadcast(rb[:], r1r[:])
                nc.vector.tensor_tensor(outT[:, h, qs], po2[:], rb[:], mul)

            for qb in range(NQB):
                qs = slice(qb * QB, (qb + 1) * QB)
                npair = qb + 1            # causal: key-chunk pairs 0..qb
                if npair <= 2:
                    # shallow rounds: batch ALL scores+exp across heads so
                    # the PE never stalls on Act inside a head
                    es_all = [[scores(qs, h, pc, npair)
                               for pc in range(npair)] for h in range(HH)]
                    for h in range(HH):
                        av_norm(qs, h, es_all[h], npair)
                else:
                    for h in range(HH):
                        es_t = [scores(qs, h, pc, npair)
                                for pc in range(npair)]
                        av_norm(qs, h, es_t, npair)
                # o_proj for the two finished 128-query blocks
                for qq in range(2):
                    qt = 2 * qb + qq
                    qsl = slice(qt * P, (qt + 1) * P)
                    ot = atmp.tile([P, 4, 512], BF, tag="ot")
                    for nt in range(4):
                        psw = psw_pool.tile([P, 512], F32, tag="psw")
                        for h in range(HH):
                            nc.tensor.matmul(psw[:], outT[:, h, qsl],
                                             wo_sb[nt][h // 4][:, h % 4, :],
                                             start=(h == 0),
                                             stop=(h == HH - 1))
                        nc.vector.tensor_copy(ot[:, nt, :], psw[:])
                        nc.sync.dma_start(
                            o_part[qt * P:(qt + 1) * P,
                                   nt * 512:(nt + 1) * 512], ot[:, nt, :])

        if dbg:
            nc.gpsimd.dma_start(dbg_outT[:], outT[:])

    nc.finalize()
    return nc


def _prep_core_inputs(inputs):
    """Shard + lay out the full inputs for the 8 cores.

    Returns a list of 8 dicts keyed by the dram tensor names above.
    """
    import ml_dtypes
    f32 = np.float32
    bf16 = ml_dtypes.bfloat16
    hs = np.ascontiguousarray(np.asarray(inputs["hidden_states"], f32))
    w_qa = np.asarray(inputs["w_qa"], f32)
    b_qa = np.asarray(inputs["b_qa"], f32)
    w_qb = np.asarray(inputs["w_qb"], f32)
    b_qb = np.asarray(inputs["b_qb"], f32)
    w_kva = np.asarray(inputs["w_kva"], f32)
    b_kva = np.asarray(inputs["b_kva"], f32)
    w_kvb = np.asarray(inputs["w_kvb"], f32)
    b_kvb = np.asarray(inputs["b_kvb"], f32)
    qn_w = np.asarray(inputs["qn_w"], f32)
    kvn_w = np.asarray(inputs["kvn_w"], f32)
    w_o = np.asarray(inputs["w_o"], f32)
    att_mask = np.asarray(inputs["attention_mask"])
    assert not b_qa.any() and not b_qb.any() and not b_kva.any() \
        and not b_kvb.any(), "nonzero projection biases not supported"

    # fold RMS-norm weights into the up-projections
    w_qb_f = qn_w[:, None] * w_qb          # [QR, H*HD]
    w_kvb_f = kvn_w[:, None] * w_kvb       # [KVR, 2*HD]

    wqa_pre = np.ascontiguousarray(
        w_qa.reshape(DK, P, QRK, P).transpose(2, 1, 0, 3)).astype(bf16)
    wkva_pre = np.ascontiguousarray(
        w_kva.reshape(DK, P, KVK, P).transpose(2, 1, 0, 3)).astype(bf16)
    wkvb_pre = np.ascontiguousarray(
        w_kvb_f.reshape(KVK, P, 2, HD).transpose(1, 2, 0, 3)).astype(bf16)

    trig_full = np.stack([
        np.asarray(inputs["cos_g"], f32),
        np.asarray(inputs["cos_l"], f32),
        np.asarray(inputs["sin_g"], f32),
        np.asarray(inputs["sin_l"], f32),
    ])  # [4, B, T, H2]
    gates_np = np.array(
        [[np.float32(inputs["nope_logit"]),
          np.float32(inputs["rope_logit"])]], f32)

    # causal 0/1 keep-masks for the two diagonal key chunks, [k, q] layout
    i = np.arange(P)[:, None]
    j = np.arange(QB)[None, :]
    dm = np.zeros((P, 2, QB), f32)
    dm[:, 0, :] = (i <= j)
    dm[:, 1, :] = (i + P <= j)
    dm = dm.astype(bf16)

    in_maps = []
    for c in range(NCORES):
        b, hh = c // 2, c % 2
        hidT = np.ascontiguousarray(
            hs[b].T.reshape(DK, P, T).astype(bf16))
        w_qb_h = w_qb_f[:, hh * HH * HD:(hh + 1) * HH * HD]
        wqb_pre = np.ascontiguousarray(
            w_qb_h.reshape(QRK, P, HH, HD).transpose(2, 1, 0, 3)).astype(bf16)
        w_o_h = w_o[hh * HH * HD:(hh + 1) * HH * HD, :]
        wo_pre = np.ascontiguousarray(
            w_o_h.reshape(2, 4, P, 4, 512).transpose(3, 0, 2, 1, 4)).astype(bf16)
        trig_b = np.ascontiguousarray(
            trig_full[:, b].transpose(0, 2, 1)).astype(bf16)
        padb_np = np.ascontiguousarray(
            np.where(att_mask[b] == 0, NEG, 0.0).astype(f32)[None, :])
        in_maps.append({
            "hid": hidT, "wqa": wqa_pre, "wkva": wkva_pre,
            "wqb": wqb_pre, "wkvb": wkvb_pre, "wo": wo_pre,
            "trig": trig_b, "gates": gates_np, "padb": padb_np,
            "dmask": dm, "onesm": np.ones((P, QB), bf16),
        })
    return in_maps


NCORES = 8


def kernel(**inputs):
    use_pad = bool((np.asarray(inputs["attention_mask"]) == 0).any())
    if use_pad not in _nc_cache:
        _nc_cache[use_pad] = build_kernel(use_pad=use_pad)
    nc = _nc_cache[use_pad]

    from concourse.bass_utils import run_bass_kernel_spmd
    in_maps = _prep_core_inputs(inputs)
    res = run_bass_kernel_spmd(nc, in_maps, core_ids=list(range(NCORES)))
    out = np.empty((B, T, D), np.float32)
    for b in range(B):
        out[b] = (res.results[2 * b]["o_part"].astype(np.float32)
                  + res.results[2 * b + 1]["o_part"].astype(np.float32))
    return out
